# revision 1
# baseline (speedup 1.0000x reference)
"""Trainium2 Bass kernel for DensePose sparse GN head (segment_reduce).

out = relu((x - mu[seg]) * rstd[seg]) * sigmoid(conv1d(segmean(relu(xn))))[seg]

Data-parallel over points across 8 NeuronCores; per-segment stats [64, C]
are computed as partial sums per shard (one-hot matmuls into PSUM) and
combined with two tiny AllReduces.

Key identities used on-device:
  xr  = relu((x - mu)*rstd)        = rstd * relu(x - mu)     (rstd > 0)
  m   = segsum(xr)/cnt             = rstd * segsum(relu(x - mu)) / cnt
  out = xr * w = relu(x*S3 + B3)   with S3 = rstd*w, B3 = -mu*rstd*w (w > 0)
"""

import sys

for _p in ("/opt/trn_rl_repo",):
    if _p not in sys.path:
        sys.path.append(_p)

import numpy as np

import concourse.bass as bass
import concourse.bacc as bacc
import concourse.mybir as mybir
import concourse.tile as tile
from concourse import masks
from concourse.bass_utils import run_bass_kernel_spmd

F32 = mybir.dt.float32
BF16 = mybir.dt.bfloat16
I32 = mybir.dt.int32
ALU = mybir.AluOpType
ACTF = mybir.ActivationFunctionType

N = 2_000_000
C = 64
NSEG = 64
EPS = 1e-5
NCORES = 8

SHARD = N // NCORES          # 250_000 real points per core
PAD = 262_144                # padded shard: 2048 tiles of 128 points
CHUNK_T = 64                 # tiles per chunk
GROUP_T = 8                  # tiles per PSUM group
PTS_PER_CHUNK = CHUNK_T * 128   # 8192
ROWS_PER_PART = PTS_PER_CHUNK // 128  # 64 consecutive rows per partition


def _consts(nc):
    """Inline (NEFF-embedded) constant tensors."""
    iota_row = np.broadcast_to(np.arange(64, dtype=np.float32), (128, 64))
    iota2_col = np.concatenate([np.arange(64), np.arange(64)]).astype(np.float32)
    from ml_dtypes import bfloat16
    c = {}
    c["iota_row_bf"] = nc.inline_tensor(
        np.ascontiguousarray(iota_row.astype(bfloat16)), name="iota_row_bf")
    c["iota2_col_f"] = nc.inline_tensor(
        np.ascontiguousarray(iota2_col[:, None]), name="iota2_col_f")
    c["ident_bf"] = nc.inline_tensor(np.eye(128, dtype=bfloat16), name="ident_bf")
    c["ones_row_f"] = nc.inline_tensor(np.ones((1, 64), np.float32),
                                       name="ones_row_f")
    return c


def build_nc(pad_n=PAD, chunk_t=CHUNK_T):
    assert pad_n % (chunk_t * 128) == 0
    n_chunks = pad_n // (chunk_t * 128)
    n_groups = chunk_t // GROUP_T
    rpp = chunk_t * 128 // 128  # rows per partition in a chunk (= chunk_t)

    nc = bacc.Bacc("TRN2", target_bir_lowering=False, debug=False,
                   num_devices=NCORES)
    x_ext = nc.declare_dram_parameter("features", [pad_n, C], F32, isOutput=False)
    idx_ext = nc.declare_dram_parameter("ins_indices_batch", [pad_n], I32,
                                        isOutput=False)
    eca_ext = nc.declare_dram_parameter("eca_weight", [1, 3], F32, isOutput=False)
    out_ext = nc.declare_dram_parameter("out", [pad_n, C], F32, isOutput=True)

    cst = _consts(nc)

    def x_chunk_ap(handle, ci):
        # [128, chunk_t*64]; partition p holds rows ci*PTS + p*rpp .. +rpp-1
        s = ci * chunk_t * 128
        return handle[s:s + chunk_t * 128, :].rearrange(
            "(p r) c -> p (r c)", p=128)

    def idx_chunk_ap(ci):
        s = ci * chunk_t * 128
        return idx_ext[s:s + chunk_t * 128].rearrange("(p r) -> p r", p=128)

    rg = [list(range(NCORES))]

    with tile.TileContext(nc) as tc:
        with (
            tc.tile_pool(name="const", bufs=1) as constp,
            tc.tile_pool(name="dram", bufs=1, space="DRAM") as dramp,
            tc.tile_pool(name="small", bufs=1) as smallp,
            tc.tile_pool(name="xin", bufs=3) as xinp,
            tc.tile_pool(name="work", bufs=2) as workp,
            tc.tile_pool(name="stage", bufs=3) as stagep,
            tc.tile_pool(name="ps_idx", bufs=3, space="PSUM") as ps_idx,
        ):
            # ---- constants into SBUF ----
            iota_row_bf = constp.tile([128, 64], BF16)
            iota2_col_f = constp.tile([128, 1], F32)
            ident_bf = constp.tile([128, 128], BF16)
            nc.sync.dma_start(iota_row_bf[:], cst["iota_row_bf"][:, :])
            nc.sync.dma_start(iota2_col_f[:], cst["iota2_col_f"][:, :])
            nc.sync.dma_start(ident_bf[:], cst["ident_bf"][:, :])

            eca_sb = constp.tile([1, 3], F32)
            nc.sync.dma_start(eca_sb[:], eca_ext[:, :])
            ones_row_f = constp.tile([1, 64], F32)
            nc.sync.dma_start(ones_row_f[:], cst["ones_row_f"][:, :])

            # ---- collective bounce buffers ----
            ar1_in = dramp.tile([64, 132], F32)
            ar1_out = dramp.tile([64, 132], F32, addr_space="Shared")
            ar2_in = dramp.tile([64, 64], F32)
            ar2_out = dramp.tile([64, 64], F32, addr_space="Shared")

            # ---- persistent small tensors ----
            mu_f = smallp.tile([64, 64], F32)
            nmu_f = smallp.tile([64, 64], F32)
            rstd_f = smallp.tile([64, 64], F32)
            rcnt = smallp.tile([64, 1], F32)
            mu_bf = smallp.tile([128, 64], BF16)   # both halves = mu
            tables3 = smallp.tile([128, 128], BF16)  # both halves = [S3||B3]
            stats_sb = smallp.tile([64, 132], F32)
            msum_sb = smallp.tile([64, 64], F32)
            eca_b = smallp.tile([64, 3], F32)

            ps_stats = tc.alloc_tile_pool(name="ps_stats", bufs=1,
                                          space="PSUM")
            ps_g2 = tc.alloc_tile_pool(name="ps_g2", bufs=3, space="PSUM")
            stats_ps = ps_stats.tile([64, 132], F32)
            m_ps = ps_stats.tile([64, 64], F32)

            def load_idx(ci):
                """Load idx for a chunk -> idxT_bf [128, chunk_t] bf16.

                idxT_bf[p, r] = idx of point (p*rpp + r) of the chunk; column
                r holds the 128 per-partition indices of "tile r".
                """
                idx_i = stagep.tile([128, rpp], I32, tag="idx_i")
                nc.sync.dma_start(idx_i[:], idx_chunk_ap(ci))
                idxT_bf = stagep.tile([128, rpp], BF16, tag="idxT_bf")
                nc.vector.tensor_copy(idxT_bf[:], idx_i[:])
                return idxT_bf

            def build_amask(idxT_bf):
                """Layout-A masks [128, chunk_t*64] bf16: A[p, r*64+s]."""
                a_bf = workp.tile([128, chunk_t * 64], BF16, tag="amask")
                in0 = idxT_bf[:].unsqueeze(2).broadcast_to([128, rpp, 64])
                in1 = iota_row_bf[:].unsqueeze(1).broadcast_to([128, rpp, 64])
                nc.vector.tensor_tensor(a_bf[:].rearrange(
                    "p (r s) -> p r s", s=64), in0, in1, ALU.is_equal)
                return a_bf

            def build_bmask(a_bf, gi):
                """Layout-B masks for one 8-tile group via PE transpose.

                Per-tile transpose(A_r [128, 64]) -> [64, 128] at base
                partition 0.  Returns bmask [64, 1024] bf16; tile
                r=8*gi+k lives at cols [128k : 128k+128].
                """
                bt_ps = ps_idx.tile([64, 1024], BF16, tag="idxB")
                for k in range(GROUP_T):
                    r = GROUP_T * gi + k
                    nc.tensor.transpose(
                        bt_ps[:, 128 * k:128 * (k + 1)],
                        a_bf[:, 64 * r:64 * (r + 1)], ident_bf[:])
                bmask = workp.tile([64, 1024], BF16, tag="bmask")
                nc.scalar.copy(bmask[:], bt_ps[:])
                return bmask

            def bslice(bmask, k_in_group):
                return bmask[:, 128 * k_in_group:128 * (k_in_group + 1)]

            # ================= PASS 1: segment stats =================
            first = True
            for ci in range(n_chunks):
                x_f = xinp.tile([128, chunk_t * 64], F32, tag="x")
                nc.sync.dma_start(x_f[:], x_chunk_ap(x_ext, ci))
                idxT_bf = load_idx(ci)
                a_bf = build_amask(idxT_bf)

                xcat = workp.tile([128, chunk_t * 132], BF16, tag="xcat")
                xcv = xcat[:].rearrange("p (r q) -> p r q", q=132)
                nc.scalar.copy(
                    xcv[:, :, 0:64],
                    x_f[:].rearrange("p (r c) -> p r c", c=64))
                nc.scalar.activation(
                    xcv[:, :, 64:128],
                    x_f[:].rearrange("p (r c) -> p r c", c=64), ACTF.Square)
                nc.vector.memset(xcv[:, :, 128:129], 1.0)

                for r in range(chunk_t):
                    nc.tensor.matmul(
                        stats_ps[:, 0:129],
                        a_bf[:, 64 * r:64 * (r + 1)],
                        xcat[:, 132 * r:132 * r + 129],
                        start=first, stop=(ci == n_chunks - 1 and
                                           r == chunk_t - 1),
                        skip_group_check=True)
                    first = False

            # ---- AllReduce #1 + derived tables ----
            nc.scalar.copy(stats_sb[:, 0:129], stats_ps[:, 0:129])
            nc.vector.memset(stats_sb[:, 129:132], 0.0)
            nc.sync.dma_start(ar1_in[:], stats_sb[:])
            nc.gpsimd.collective_compute(
                "AllReduce", ALU.add, replica_groups=rg,
                ins=[ar1_in[:]], outs=[ar1_out[:]])
            nc.sync.dma_start(stats_sb[:], ar1_out[:])

            cnt_c = smallp.tile([64, 1], F32)
            nc.vector.tensor_scalar(cnt_c[:], stats_sb[:, 128:129], 1.0, None,
                                    ALU.max)
            nc.vector.reciprocal(rcnt[:], cnt_c[:])
            nc.vector.tensor_scalar(mu_f[:], stats_sb[:, 0:64], rcnt[:], None,
                                    ALU.mult)
            nc.vector.tensor_scalar(nmu_f[:], mu_f[:], -1.0, None, ALU.mult)
            es2 = smallp.tile([64, 64], F32)
            nc.vector.tensor_scalar(es2[:], stats_sb[:, 64:128], rcnt[:], None,
                                    ALU.mult)
            var_f = smallp.tile([64, 64], F32)
            nc.vector.tensor_tensor(var_f[:], mu_f[:], mu_f[:], ALU.mult)
            nc.vector.tensor_tensor(var_f[:], es2[:], var_f[:], ALU.subtract)
            nc.vector.tensor_scalar(var_f[:], var_f[:], float(EPS), None,
                                    ALU.add)
            sd_f = smallp.tile([64, 64], F32)
            nc.scalar.sqrt(sd_f[:], var_f[:])
            nc.vector.reciprocal(rstd_f[:], sd_f[:])
            nc.vector.tensor_copy(mu_bf[0:64, :], mu_f[:])
            nc.sync.dma_start(mu_bf[64:128, :], mu_bf[0:64, :])

            # ================= PASS 2: m = segsum(relu(x - mu)) =================
            first = True
            for ci in range(n_chunks):
                x_f = xinp.tile([128, chunk_t * 64], F32, tag="x")
                nc.sync.dma_start(x_f[:], x_chunk_ap(x_ext, ci))
                idxT_bf = load_idx(ci)
                a_bf = build_amask(idxT_bf)
                u_bf = workp.tile([128, chunk_t * 64], BF16, tag="u")

                for gi in range(n_groups):
                    bmask = build_bmask(a_bf, gi)
                    gps = ps_g2.tile([128, 512], F32, tag="g2", name="g2")
                    for k in range(GROUP_T):
                        nc.tensor.matmul(
                            gps[:, 64 * k:64 * (k + 1)],
                            bslice(bmask, k), mu_bf[0:64, :],
                            start=True, stop=True, skip_group_check=True)
                    us = stagep.tile([128, 512], F32, tag="us")
                    lo, hi = 512 * gi, 512 * (gi + 1)
                    nc.vector.tensor_tensor(us[:], x_f[:, lo:hi], gps[:],
                                            ALU.subtract)
                    nc.scalar.activation(u_bf[:, lo:hi], us[:], ACTF.Relu)
                    for k in range(GROUP_T):
                        r = GROUP_T * gi + k
                        nc.tensor.matmul(
                            m_ps[:],
                            a_bf[:, 64 * r:64 * (r + 1)],
                            u_bf[:, 64 * r:64 * (r + 1)],
                            start=first, stop=(ci == n_chunks - 1 and
                                               r == chunk_t - 1),
                            skip_group_check=True)
                        first = False

            # ---- AllReduce #2 + ECA tables ----
            nc.scalar.copy(msum_sb[:], m_ps[:])
            nc.sync.dma_start(ar2_in[:], msum_sb[:])
            nc.gpsimd.collective_compute(
                "AllReduce", ALU.add, replica_groups=rg,
                ins=[ar2_in[:]], outs=[ar2_out[:]])
            nc.sync.dma_start(msum_sb[:], ar2_out[:])

            m_f = smallp.tile([64, 64], F32)
            nc.vector.tensor_scalar(m_f[:], msum_sb[:], rcnt[:], None, ALU.mult)
            nc.vector.tensor_tensor(m_f[:], m_f[:], rstd_f[:], ALU.mult)

            mpad = smallp.tile([64, 66], F32)
            nc.vector.memset(mpad[:, 0:1], 0.0)
            nc.vector.memset(mpad[:, 65:66], 0.0)
            nc.vector.tensor_copy(mpad[:, 1:65], m_f[:])
            eca_ps = ps_g2.tile([64, 3], F32, tag="g2", name="eca_ps")
            nc.tensor.matmul(eca_ps[:], ones_row_f[:], eca_sb[:],
                             start=True, stop=True, skip_group_check=True)
            nc.scalar.copy(eca_b[:], eca_ps[:])
            cv = smallp.tile([64, 64], F32)
            nc.vector.tensor_scalar(cv[:], mpad[:, 1:65], eca_b[:, 1:2], None,
                                    ALU.mult)
            nc.vector.scalar_tensor_tensor(cv[:], mpad[:, 0:64], eca_b[:, 0:1],
                                           cv[:], ALU.mult, ALU.add)
            nc.vector.scalar_tensor_tensor(cv[:], mpad[:, 2:66], eca_b[:, 2:3],
                                           cv[:], ALU.mult, ALU.add)
            w_f = smallp.tile([64, 64], F32)
            nc.scalar.activation(w_f[:], cv[:], ACTF.Sigmoid)
            s3_f = smallp.tile([64, 64], F32)
            nc.vector.tensor_tensor(s3_f[:], rstd_f[:], w_f[:], ALU.mult)
            nc.vector.tensor_copy(tables3[0:64, 0:64], s3_f[:])
            nc.vector.tensor_tensor(tables3[0:64, 64:128], nmu_f[:], s3_f[:],
                                    ALU.mult)
            nc.sync.dma_start(tables3[64:128, :], tables3[0:64, :])

            ps_g2.release()
            ps_stats.release()
            ps_g3 = tc.alloc_tile_pool(name="ps_g3", bufs=2, space="PSUM")

            # ================= PASS 3: out = relu(x*S3 + B3) =================
            for ci in range(n_chunks):
                x_f = xinp.tile([128, chunk_t * 64], F32, tag="x")
                nc.sync.dma_start(x_f[:], x_chunk_ap(x_ext, ci))
                idxT_bf = load_idx(ci)
                a_bf = build_amask(idxT_bf)
                ost = workp.tile([128, chunk_t * 64], F32, tag="ost")

                for gi in range(n_groups):
                    bmask = build_bmask(a_bf, gi)
                    gps = ps_g3.tile([128, 1024], F32, tag="g3", name="g3")
                    for k in range(GROUP_T):
                        nc.tensor.matmul(
                            gps[:, 128 * k:128 * (k + 1)],
                            bslice(bmask, k), tables3[0:64, :],
                            start=True, stop=True, skip_group_check=True)
                    sg = gps[:].rearrange("p (t q) -> p t q", q=128)
                    if gi % 2 == 0:
                        xs = stagep.tile([128, 1024], F32, tag="xs")
                    lo, hi = 512 * gi, 512 * (gi + 1)
                    xlo = 512 * (gi % 2)
                    xsv = xs[:, xlo:xlo + 512]
                    nc.vector.tensor_tensor(
                        xsv.rearrange("p (t c) -> p t c", c=64),
                        x_f[:, lo:hi].rearrange("p (t c) -> p t c", c=64),
                        sg[:, :, 0:64], ALU.mult)
                    nc.vector.tensor_tensor(
                        xsv.rearrange("p (t c) -> p t c", c=64),
                        xsv.rearrange("p (t c) -> p t c", c=64),
                        sg[:, :, 64:128], ALU.add)
                    if gi % 2 == 1:
                        nc.scalar.activation(ost[:, hi - 1024:hi], xs[:],
                                             ACTF.Relu)

                nc.sync.dma_start(x_chunk_ap(out_ext, ci), ost[:])

            ps_g3.release()

    nc.compile()
    return nc


_cache = {}


def _get_nc(pad_n, chunk_t):
    key = (pad_n, chunk_t)
    if key not in _cache:
        _cache[key] = build_nc(pad_n, chunk_t)
    return _cache[key]


last_result = None


def _install_ntff_hook():
    """Provide antenv.axon_hooks (missing in this image) so
    run_bass_kernel_spmd(trace=True) can reach the axon NTFF profiler."""
    import types

    try:
        from antenv.axon_hooks import get_axon_ntff_profile_hook  # noqa: F401
        return
    except ImportError:
        pass
    if "/root/.axon_site" not in sys.path:
        sys.path.insert(0, "/root/.axon_site")
    from trn_agent_boot.trn_boot import _ntff_profile_via_ctypes
    hook = _ntff_profile_via_ctypes("/opt/axon/libaxon_pjrt.so")
    try:
        import antenv
    except ImportError:
        antenv = types.ModuleType("antenv")
        sys.modules["antenv"] = antenv
    mod = types.ModuleType("antenv.axon_hooks")
    mod.get_axon_ntff_profile_hook = lambda: hook
    mod.set_axon_ntff_profile_hook = lambda h: None
    sys.modules["antenv.axon_hooks"] = mod
    antenv.axon_hooks = mod
    import concourse.bass_utils as _bu
    _bu.upload_artifacts = lambda d: "local://" + str(d)


def kernel(features, ins_indices_batch, eca_weight, _pad=PAD, _chunk_t=CHUNK_T,
           _trace=False):
    global last_result
    features = np.asarray(features, np.float32)
    ins_indices_batch = np.asarray(ins_indices_batch, np.int32)
    eca = np.asarray(eca_weight, np.float32).reshape(1, 3)
    n = features.shape[0]
    shard = n // NCORES
    assert shard * NCORES == n

    nc = _get_nc(_pad, _chunk_t)
    in_maps = []
    for i in range(NCORES):
        xs = features[i * shard:(i + 1) * shard]
        ii = ins_indices_batch[i * shard:(i + 1) * shard]
        xp = np.zeros((_pad, C), np.float32)
        xp[:shard] = xs
        ip = np.full((_pad,), NSEG, np.int32)  # pad idx -> matches no segment
        ip[:shard] = ii
        in_maps.append({"features": xp, "ins_indices_batch": ip,
                        "eca_weight": eca})

    if _trace:
        _install_ntff_hook()
    try:
        res = run_bass_kernel_spmd(nc, in_maps, core_ids=list(range(NCORES)),
                                   trace=_trace)
    except Exception:
        if not _trace:
            raise
        import traceback
        traceback.print_exc()
        print("traced run failed; falling back to untraced", flush=True)
        res = run_bass_kernel_spmd(nc, in_maps, core_ids=list(range(NCORES)))
    last_result = res
    outs = [res.results[i]["out"][:shard] for i in range(NCORES)]
    return np.concatenate(outs, axis=0)


if __name__ == "__main__":
    rng = np.random.default_rng(0)
    n_test = NCORES * 2 * CHUNK_T * 128
    x = rng.standard_normal((n_test, C), dtype=np.float32)
    ii = rng.integers(0, NSEG, n_test).astype(np.int32)
    k = (rng.standard_normal((1, 1, 3)) * 0.1).astype(np.float32)
    out = kernel(x, ii, k, _pad=2 * CHUNK_T * 128)
    print("out", out.shape, out.dtype, float(np.abs(out).mean()))



# revision 4
# speedup vs baseline: 2.4300x; 2.4300x over previous
"""Trainium2 Bass kernel for DensePose sparse GN head (segment_reduce).

out = relu((x - mu[seg]) * rstd[seg]) * sigmoid(conv1d(segmean(relu(xn))))[seg]

Strategy: the host pre-sorts points by segment id and deals them evenly
across the 8 cores so that every (core, segment) run has an identical
length LP (padded with duplicated points of the same segment; counts then
become the compile-time constant 8*LP).  Data is uploaded in f16,
channel-major, packed two point-halves on the 128 partitions:

    x_dev[half*64 + ch, s*LP2 + t] = x[pt(core, s, 2*t + half), ch]

On device every segment is a contiguous column range, so segment sums are
free-dim reductions (DVE tensor_scalar accumulate / Act accumulate) and
the normalize+ECA scaling is a per-partition scale/bias elementwise op.
No masks, no gathers.  The ECA conv over channels (= partitions) is one
tiny tridiagonal-band 64x64 PE matmul.  Two tiny AllReduces combine the
per-core partial stats.

Identities used on-device (w > 0, rstd > 0):
  xr  = relu((x - mu)*rstd) = rstd*relu(x - mu)
  m   = rstd * segsum(relu(x - mu)) / cnt
  out = relu(x*S3 + B3)  with  S3 = rstd*w,  B3 = -mu*rstd*w
  segsum(relu(x - mu)) = segsum(max(x, mu)) - cnt*mu
"""

import sys

for _p in ("/opt/trn_rl_repo",):
    if _p not in sys.path:
        sys.path.append(_p)

import numpy as np

import concourse.bass as bass
import concourse.bacc as bacc
import concourse.mybir as mybir
import concourse.tile as tile
from concourse.bass_utils import run_bass_kernel_spmd

F32 = mybir.dt.float32
F16 = mybir.dt.float16
ALU = mybir.AluOpType
ACTF = mybir.ActivationFunctionType

C = 64
NSEG = 64
EPS = 1e-5
NCORES = 8
GSEG = 4          # segments per DMA group


def _consts(nc):
    ii = np.vstack([np.eye(64, dtype=np.float32)] * 2)          # [128, 64]
    band3 = np.zeros((64, 192), np.float32)
    for j, off in enumerate((-1, 0, 1)):
        for c in range(64):
            cp = c + off
            if 0 <= cp < 64:
                band3[cp, 64 * j + c] = 1.0
    ones_row = np.ones((1, 64), np.float32)
    c = {}
    c["ii"] = nc.inline_tensor(np.ascontiguousarray(ii), name="ii_f")
    c["band3"] = nc.inline_tensor(np.ascontiguousarray(band3), name="band3_f")
    c["ones_row"] = nc.inline_tensor(ones_row, name="ones_row_f")
    return c


def build_nc(lp2):
    ncols = NSEG * lp2
    glen = GSEG * lp2
    ngroups = NSEG // GSEG
    rcnt = 1.0 / float(NCORES * 2 * lp2)     # 1 / (8 * LP)

    nc = bacc.Bacc("TRN2", target_bir_lowering=False, debug=False,
                   num_devices=NCORES)
    x_ext = nc.declare_dram_parameter("xT", [128, ncols], F16, isOutput=False)
    eca_ext = nc.declare_dram_parameter("eca_weight", [1, 3], F32,
                                        isOutput=False)
    out_ext = nc.declare_dram_parameter("out", [128, ncols], F16,
                                        isOutput=True)
    cst = _consts(nc)
    rg = [list(range(NCORES))]

    with tile.TileContext(nc) as tc:
        with (
            tc.tile_pool(name="const", bufs=1) as constp,
            tc.tile_pool(name="dram", bufs=1, space="DRAM") as dramp,
            tc.tile_pool(name="small", bufs=1) as smallp,
            tc.tile_pool(name="xin", bufs=3) as xinp,
            tc.tile_pool(name="scra", bufs=2) as scrap,
            tc.tile_pool(name="scrd", bufs=2) as scrdp,
            tc.tile_pool(name="outp", bufs=3) as outp,
            tc.tile_pool(name="ps", bufs=1, space="PSUM") as psp,
        ):
            # ---- constants ----
            ii_sb = constp.tile([128, 64], F32)
            band3_sb = constp.tile([64, 192], F32)
            ones_sb = constp.tile([1, 64], F32)
            eca_sb = constp.tile([1, 3], F32)
            nc.sync.dma_start(ii_sb[:], cst["ii"][:, :])
            nc.sync.dma_start(band3_sb[:], cst["band3"][:, :])
            nc.sync.dma_start(ones_sb[:], cst["ones_row"][:, :])
            nc.sync.dma_start(eca_sb[:], eca_ext[:, :])

            # ---- collective bounce buffers ----
            ar1_in = dramp.tile([64, 128], F32)
            ar1_out = dramp.tile([64, 128], F32, addr_space="Shared")
            ar2_in = dramp.tile([64, 64], F32)
            ar2_out = dramp.tile([64, 64], F32, addr_space="Shared")

            # ---- persistent small tensors ----
            s1_T = smallp.tile([128, 64], F32)    # sum(x) per (half*ch, seg)
            s2_T = smallp.tile([128, 64], F32)    # sum(x^2)
            msA_T = smallp.tile([128, 64], F32)   # sum(max(x, mu))
            msC_T = smallp.tile([128, 64], F32)   # corrected relu-sum
            g1_sb = smallp.tile([64, 128], F32)
            g2_sb = smallp.tile([64, 64], F32)
            fold_sb = smallp.tile([64, 128], F32)
            fold2_sb = smallp.tile([64, 64], F32)
            mu_T = smallp.tile([64, 64], F32)     # [ch, seg]
            es2 = smallp.tile([64, 64], F32)
            var_T = smallp.tile([64, 64], F32)
            sd_T = smallp.tile([64, 64], F32)
            rstd_T = smallp.tile([64, 64], F32)
            m_T = smallp.tile([64, 64], F32)
            w_T = smallp.tile([64, 64], F32)
            s3_T = smallp.tile([64, 64], F32)
            b3_T = smallp.tile([64, 64], F32)
            band_sb = smallp.tile([64, 64], F32)
            eca_b = smallp.tile([64, 3], F32)
            mu_pack = smallp.tile([128, 64], F32)
            scale_pack = smallp.tile([128, 64], F32)
            bias_pack = smallp.tile([128, 64], F32)
            eps_col = smallp.tile([64, 1], F32)
            nc.vector.memset(eps_col[:], float(EPS))

            # ---- ECA weight broadcast + band matrix (input-only deps) ----
            eca_ps = psp.tile([64, 3], F32, tag="eca")
            nc.tensor.matmul(eca_ps[:], ones_sb[:], eca_sb[:],
                             start=True, stop=True, skip_group_check=True)
            nc.scalar.copy(eca_b[:], eca_ps[:])
            nc.vector.tensor_scalar(band_sb[:], band3_sb[:, 0:64],
                                    eca_b[:, 0:1], None, ALU.mult)
            nc.vector.scalar_tensor_tensor(band_sb[:], band3_sb[:, 64:128],
                                           eca_b[:, 1:2], band_sb[:],
                                           ALU.mult, ALU.add)
            nc.vector.scalar_tensor_tensor(band_sb[:], band3_sb[:, 128:192],
                                           eca_b[:, 2:3], band_sb[:],
                                           ALU.mult, ALU.add)

            # ================= PASS 1: sum(x), sum(x^2) =================
            for g in range(ngroups):
                xt = xinp.tile([128, glen], F16, tag="x")
                nc.sync.dma_start(xt[:], x_ext[:, g * glen:(g + 1) * glen])
                sa = scrap.tile([128, glen], F16, tag="sa")
                sd2 = scrdp.tile([128, glen], F16, tag="sd")
                for k in range(GSEG):
                    s = g * GSEG + k
                    lo, hi = k * lp2, (k + 1) * lp2
                    nc.scalar.activation(sa[:, lo:hi], xt[:, lo:hi],
                                         ACTF.Square,
                                         accum_out=s2_T[:, s:s + 1])
                    nc.vector.tensor_scalar(sd2[:, lo:hi], xt[:, lo:hi],
                                            0.0, None, ALU.add, op1=ALU.add,
                                            accum_out=s1_T[:, s:s + 1])

            # ---- fold halves + AllReduce #1 + derived stats ----
            fold_ps = psp.tile([64, 128], F32, tag="fold1")
            nc.tensor.matmul(fold_ps[:, 0:64], ii_sb[:], s1_T[:],
                             start=True, stop=True, skip_group_check=True)
            nc.tensor.matmul(fold_ps[:, 64:128], ii_sb[:], s2_T[:],
                             start=True, stop=True, skip_group_check=True)
            nc.scalar.copy(fold_sb[:], fold_ps[:])
            nc.sync.dma_start(ar1_in[:], fold_sb[:])
            nc.gpsimd.collective_compute(
                "AllReduce", ALU.add, replica_groups=rg,
                ins=[ar1_in[:]], outs=[ar1_out[:]])
            nc.sync.dma_start(g1_sb[:], ar1_out[:])

            nc.vector.tensor_scalar(mu_T[:], g1_sb[:, 0:64], rcnt, None,
                                    ALU.mult)
            nc.vector.tensor_scalar(es2[:], g1_sb[:, 64:128], rcnt, None,
                                    ALU.mult)
            nc.vector.tensor_tensor(var_T[:], mu_T[:], mu_T[:], ALU.mult)
            nc.vector.scalar_tensor_tensor(var_T[:], var_T[:], -1.0, es2[:],
                                           ALU.mult, ALU.add)
            nc.scalar.activation(sd_T[:], var_T[:], ACTF.Sqrt,
                                 bias=eps_col[:])
            nc.vector.reciprocal(rstd_T[:], sd_T[:])
            nc.vector.tensor_copy(mu_pack[0:64, :], mu_T[:])
            nc.sync.dma_start(mu_pack[64:128, :], mu_pack[0:64, :])

            # ================= PASS 2: sum(max(x, mu)) =================
            for g in range(ngroups):
                xt = xinp.tile([128, glen], F16, tag="x")
                nc.sync.dma_start(xt[:], x_ext[:, g * glen:(g + 1) * glen])
                sd2 = scrdp.tile([128, glen], F16, tag="sd")
                for k in range(GSEG):
                    s = g * GSEG + k
                    lo, hi = k * lp2, (k + 1) * lp2
                    nc.vector.tensor_scalar(sd2[:, lo:hi], xt[:, lo:hi],
                                            mu_pack[:, s:s + 1], None,
                                            ALU.max, op1=ALU.add,
                                            accum_out=msA_T[:, s:s + 1])

            # relu-sum = sum(max(x,mu)) - LP2*mu   (per half)
            nc.vector.scalar_tensor_tensor(msC_T[:], mu_pack[:],
                                           float(-lp2), msA_T[:],
                                           ALU.mult, ALU.add)

            # ---- fold + AllReduce #2 + ECA tables ----
            fold2_ps = psp.tile([64, 64], F32, tag="fold2")
            nc.tensor.matmul(fold2_ps[:], ii_sb[:], msC_T[:],
                             start=True, stop=True, skip_group_check=True)
            nc.scalar.copy(fold2_sb[:], fold2_ps[:])
            nc.sync.dma_start(ar2_in[:], fold2_sb[:])
            nc.gpsimd.collective_compute(
                "AllReduce", ALU.add, replica_groups=rg,
                ins=[ar2_in[:]], outs=[ar2_out[:]])
            nc.sync.dma_start(g2_sb[:], ar2_out[:])

            nc.vector.tensor_scalar(m_T[:], g2_sb[:], rcnt, None, ALU.mult)
            nc.vector.tensor_tensor(m_T[:], m_T[:], rstd_T[:], ALU.mult)
            conv_ps = psp.tile([64, 64], F32, tag="conv")
            nc.tensor.matmul(conv_ps[:], band_sb[:], m_T[:],
                             start=True, stop=True, skip_group_check=True)
            nc.scalar.activation(w_T[:], conv_ps[:], ACTF.Sigmoid)
            nc.vector.tensor_tensor(s3_T[:], rstd_T[:], w_T[:], ALU.mult)
            nc.vector.scalar_tensor_tensor(b3_T[:], mu_T[:], -1.0, s3_T[:],
                                           ALU.mult, ALU.mult)
            nc.vector.tensor_copy(scale_pack[0:64, :], s3_T[:])
            nc.sync.dma_start(scale_pack[64:128, :], scale_pack[0:64, :])
            nc.vector.tensor_copy(bias_pack[0:64, :], b3_T[:])
            nc.sync.dma_start(bias_pack[64:128, :], bias_pack[0:64, :])

            # ================= PASS 3: out = relu(x*S3 + B3) ============
            for g in range(ngroups):
                xt = xinp.tile([128, glen], F16, tag="x")
                nc.sync.dma_start(xt[:], x_ext[:, g * glen:(g + 1) * glen])
                ot = outp.tile([128, glen], F16, tag="o")
                for k in range(GSEG):
                    s = g * GSEG + k
                    lo, hi = k * lp2, (k + 1) * lp2
                    nc.vector.tensor_scalar(ot[:, lo:hi], xt[:, lo:hi],
                                            scale_pack[:, s:s + 1],
                                            bias_pack[:, s:s + 1],
                                            ALU.mult, op1=ALU.add)
                    nc.vector.tensor_scalar(ot[:, lo:hi], ot[:, lo:hi],
                                            0.0, None, ALU.max)
                nc.sync.dma_start(out_ext[:, g * glen:(g + 1) * glen], ot[:])

    nc.compile()
    return nc


_cache = {}


def _get_nc(lp2):
    if lp2 not in _cache:
        _cache[lp2] = build_nc(lp2)
    return _cache[lp2]


last_result = None


def _install_ntff_hook():
    """Provide antenv.axon_hooks (missing in this image) so
    run_bass_kernel_spmd(trace=True) can reach the axon NTFF profiler."""
    import types

    try:
        from antenv.axon_hooks import get_axon_ntff_profile_hook  # noqa: F401
        return
    except ImportError:
        pass
    if "/root/.axon_site" not in sys.path:
        sys.path.insert(0, "/root/.axon_site")
    from trn_agent_boot.trn_boot import _ntff_profile_via_ctypes
    hook = _ntff_profile_via_ctypes("/opt/axon/libaxon_pjrt.so")
    try:
        import antenv
    except ImportError:
        antenv = types.ModuleType("antenv")
        sys.modules["antenv"] = antenv
    mod = types.ModuleType("antenv.axon_hooks")
    mod.get_axon_ntff_profile_hook = lambda: hook
    mod.set_axon_ntff_profile_hook = lambda h: None
    sys.modules["antenv.axon_hooks"] = mod
    antenv.axon_hooks = mod
    import concourse.bass_utils as _bu
    _bu.upload_artifacts = lambda d: "local://" + str(d)


def _prep_inputs(x, idx, eca):
    """Sort by segment, deal evenly over cores, pad each (core, seg) run
    to the common even length LP with duplicated points.  Returns
    (in_maps, grids, lp2)."""
    n = x.shape[0]
    order = np.argsort(idx, kind="stable")
    counts = np.bincount(idx, minlength=NSEG).astype(np.int64)
    starts = np.zeros(NSEG + 1, np.int64)
    starts[1:] = np.cumsum(counts)

    q, r = np.divmod(counts, NCORES)
    maxchunk = int((q + (r > 0).astype(np.int64)).max())
    lp = max(2, ((maxchunk + 1) // 2) * 2)       # even
    lp2 = lp // 2

    grids = []
    in_maps = []
    for kcore in range(NCORES):
        grid = np.empty((NSEG, lp), np.int64)
        for s in range(NSEG):
            n_s = counts[s]
            run = order[starts[s]:starts[s] + n_s]
            qq, rr = divmod(int(n_s), NCORES)
            a = kcore * qq + min(kcore, rr)
            b = a + qq + (1 if kcore < rr else 0)
            chunk = run[a:b]
            assert chunk.size > 0, f"empty (core,seg)=({kcore},{s})"
            grid[s] = np.resize(chunk, lp)
        grids.append(grid)
        xg = x[grid.reshape(-1)].reshape(NSEG, lp2, 2, C)
        dev = np.ascontiguousarray(
            xg.transpose(2, 3, 0, 1).reshape(128, NSEG * lp2),
            dtype=np.float16)
        in_maps.append({"xT": dev, "eca_weight": eca})
    return in_maps, grids, lp2


def kernel(features, ins_indices_batch, eca_weight, _trace=False):
    global last_result
    x = np.asarray(features, np.float32)
    idx = np.asarray(ins_indices_batch, np.int32)
    eca = np.asarray(eca_weight, np.float32).reshape(1, 3)
    n = x.shape[0]

    in_maps, grids, lp2 = _prep_inputs(x, idx, eca)
    nc = _get_nc(lp2)

    if _trace:
        _install_ntff_hook()
    try:
        res = run_bass_kernel_spmd(nc, in_maps, core_ids=list(range(NCORES)),
                                   trace=_trace)
    except Exception:
        if not _trace:
            raise
        import traceback
        traceback.print_exc()
        print("traced run failed; falling back to untraced", flush=True)
        res = run_bass_kernel_spmd(nc, in_maps, core_ids=list(range(NCORES)))
    last_result = res

    out = np.empty((n, C), np.float32)
    lp = 2 * lp2
    for kcore in range(NCORES):
        od = res.results[kcore]["out"]            # [128, NSEG*lp2] f16
        vals = od.reshape(2, C, NSEG, lp2).transpose(2, 3, 0, 1)
        out[grids[kcore].reshape(-1)] = vals.reshape(NSEG * lp, C)
    return out


if __name__ == "__main__":
    rng = np.random.default_rng(0)
    n_test = 200_000
    x = rng.standard_normal((n_test, C), dtype=np.float32)
    ii = rng.integers(0, NSEG, n_test).astype(np.int32)
    k = (rng.standard_normal((1, 1, 3)) * 0.1).astype(np.float32)
    out = kernel(x, ii, k)

    # numpy reference
    seg = ii
    cnt = np.maximum(np.bincount(seg, minlength=NSEG), 1).astype(np.float64)
    s = np.zeros((NSEG, C)); np.add.at(s, seg, x.astype(np.float64))
    s2 = np.zeros((NSEG, C)); np.add.at(s2, seg, x.astype(np.float64) ** 2)
    mu = s / cnt[:, None]
    var = s2 / cnt[:, None] - mu ** 2
    xn = (x - mu[seg]) / np.sqrt(var[seg] + EPS)
    xr = np.maximum(xn, 0)
    m = np.zeros((NSEG, C)); np.add.at(m, seg, xr)
    m = m / cnt[:, None]
    kf = k.reshape(3)
    mp = np.pad(m, ((0, 0), (1, 1)))
    conv = kf[0] * mp[:, 0:64] + kf[1] * mp[:, 1:65] + kf[2] * mp[:, 2:66]
    w = 1.0 / (1.0 + np.exp(-conv))
    exp = xr * w[seg]
    err = np.linalg.norm(out - exp) / np.linalg.norm(exp)
    print("out", out.shape, out.dtype, "rel_err", err)


# revision 12
# speedup vs baseline: 2.6723x; 1.0997x over previous
"""Trainium2 Bass kernel for DensePose sparse GN head (segment_reduce).

out = relu((x - mu[seg]) * rstd[seg]) * sigmoid(conv1d(segmean(relu(xn))))[seg]

Host pre-sorts points by segment id and deals them evenly across the 8
cores so every (core, segment) run has identical length LP (padded with
duplicated points; counts become the compile-time constant 8*LP).  Data
is f16, channel-major, two point-halves packed on 128 partitions:

    x_dev[half*64 + ch, s*LP2 + t] = x[pt(core, s, 2*t + half), ch]

Every segment is a contiguous column range: segment sums are free-dim
reductions (DVE tensor_tensor_reduce / Act accumulate), normalize+ECA
scaling is per-partition scale/bias.  The ECA conv over channels
(= partitions) is a tiny tridiagonal-band 64x64 PE matmul.

The computation is separable per segment, so segments are processed in
NBLK pipelined blocks: pass1(b) -> AllReduce1(b) -> pass2(b) ->
AllReduce2(b) -> pass3(b), with blocks interleaved so the collectives
hide behind other blocks' streaming.

Identities (w > 0, rstd > 0):
  out = relu(x*S3 + B3),  S3 = rstd*w,  B3 = -mu*rstd*w
  segsum(relu(x - mu)) = segsum(max(x, mu)) - cnt*mu
"""

import sys

for _p in ("/opt/trn_rl_repo",):
    if _p not in sys.path:
        sys.path.append(_p)

import numpy as np

import concourse.bass as bass
import concourse.bacc as bacc
import concourse.mybir as mybir
import concourse.tile as tile
from concourse.bass_utils import run_bass_kernel_spmd

F32 = mybir.dt.float32
F16 = mybir.dt.float16
ALU = mybir.AluOpType
ACTF = mybir.ActivationFunctionType

C = 64
NSEG = 64
EPS = 1e-5
NCORES = 8
GSEG = 4           # segments per DMA group
NBLK = 4           # pipeline blocks
BSEG = NSEG // NBLK   # segments per block (16)

# engine split knobs (per 16-seg block); sum(x^2) always runs on Act
P1_DVE = 9         # segs whose sum(x) runs on DVE (rest on Act copy+accum)
P2_DVE = 9         # segs whose relu-sum runs on DVE (rest on Act relu+accum)
P3_DVE = 16        # segs whose output op runs on DVE (rest on Act relu)


def _consts(nc):
    ii = np.vstack([np.eye(64, dtype=np.float32)] * 2)          # [128, 64]
    band3 = np.zeros((64, 192), np.float32)
    for j, off in enumerate((-1, 0, 1)):
        for c in range(64):
            cp = c + off
            if 0 <= cp < 64:
                band3[cp, 64 * j + c] = 1.0
    ones_row = np.ones((1, 64), np.float32)
    c = {}
    c["ii"] = nc.inline_tensor(np.ascontiguousarray(ii), name="ii_f")
    c["band3"] = nc.inline_tensor(np.ascontiguousarray(band3), name="band3_f")
    c["ones_row"] = nc.inline_tensor(ones_row, name="ones_row_f")
    return c


def build_nc(lp2):
    ncols = NSEG * lp2
    glen = GSEG * lp2
    blen = BSEG * lp2
    gpb = BSEG // GSEG           # DMA groups per block
    rcnt = 1.0 / float(NCORES * 2 * lp2)     # 1 / (8 * LP)

    nc = bacc.Bacc("TRN2", target_bir_lowering=False, debug=False,
                   num_devices=NCORES)
    x_ext = nc.declare_dram_parameter("xT", [128, ncols], F16, isOutput=False)
    eca_ext = nc.declare_dram_parameter("eca_weight", [1, 3], F32,
                                        isOutput=False)
    out_ext = nc.declare_dram_parameter("out", [128, ncols], F16,
                                        isOutput=True)
    cst = _consts(nc)
    rg = [list(range(NCORES))]

    with tile.TileContext(nc) as tc:
        with (
            tc.tile_pool(name="const", bufs=1) as constp,
            tc.tile_pool(name="dram", bufs=1, space="DRAM") as dramp,
            tc.tile_pool(name="small", bufs=1) as smallp,
            tc.tile_pool(name="xin", bufs=3) as xinp,
            tc.tile_pool(name="scra", bufs=2) as scrap,
            tc.tile_pool(name="scrd", bufs=2) as scrdp,
            tc.tile_pool(name="outp", bufs=3) as outp,
            tc.tile_pool(name="ps", bufs=2, space="PSUM") as psp,
        ):
            # ---- constants ----
            ii_sb = constp.tile([128, 64], F32)
            band3_sb = constp.tile([64, 192], F32)
            ones_sb = constp.tile([1, 64], F32)
            eca_sb = constp.tile([1, 3], F32)
            nc.sync.dma_start(ii_sb[:], cst["ii"][:, :])
            nc.sync.dma_start(band3_sb[:], cst["band3"][:, :])
            nc.sync.dma_start(ones_sb[:], cst["ones_row"][:, :])
            nc.sync.dma_start(eca_sb[:], eca_ext[:, :])

            # ---- collective bounce buffers (per block) ----
            ar1_in = [dramp.tile([64, 2 * BSEG], F32, name=f"ar1_in{b}")
                      for b in range(NBLK)]
            ar1_out = [dramp.tile([64, 2 * BSEG], F32, addr_space="Shared",
                                  name=f"ar1_out{b}") for b in range(NBLK)]
            ar2_in = [dramp.tile([64, BSEG], F32, name=f"ar2_in{b}")
                      for b in range(NBLK)]
            ar2_out = [dramp.tile([64, BSEG], F32, addr_space="Shared",
                                  name=f"ar2_out{b}") for b in range(NBLK)]

            # ---- persistent small tensors ----
            s1_T = smallp.tile([128, 64], F32)    # sum(x) per (half*ch, seg)
            s2_T = smallp.tile([128, 64], F32)    # sum(x^2)
            msC_T = smallp.tile([128, 64], F32)   # relu-sum (corrected)
            g1_sb = smallp.tile([64, 2 * 64], F32)
            g2_sb = smallp.tile([64, 64], F32)
            fold_sb = smallp.tile([64, 2 * 64], F32)
            fold2_sb = smallp.tile([64, 64], F32)
            mu_T = smallp.tile([64, 64], F32)     # [ch, seg]
            es2 = smallp.tile([64, 64], F32)
            var_T = smallp.tile([64, 64], F32)
            sd_T = smallp.tile([64, 64], F32)
            rstd_T = smallp.tile([64, 64], F32)
            m_T = smallp.tile([64, 64], F32)
            w_T = smallp.tile([64, 64], F32)
            s3_T = smallp.tile([64, 64], F32)
            b3_T = smallp.tile([64, 64], F32)
            band_sb = smallp.tile([64, 64], F32)
            eca_b = smallp.tile([64, 3], F32)
            mu_pack = smallp.tile([128, 64], F32)
            nmu_pack = smallp.tile([128, 64], F32)
            scale_pack = smallp.tile([128, 64], F32)
            bias_pack = smallp.tile([128, 64], F32)
            eps_col = smallp.tile([64, 1], F32)
            nc.vector.memset(eps_col[:], float(EPS))

            # ---- ECA weight broadcast + band matrix (input-only deps) ----
            eca_ps = psp.tile([64, 3], F32, tag="eca")
            nc.tensor.matmul(eca_ps[:], ones_sb[:], eca_sb[:],
                             start=True, stop=True, skip_group_check=True)
            nc.scalar.copy(eca_b[:], eca_ps[:])
            nc.vector.tensor_scalar(band_sb[:], band3_sb[:, 0:64],
                                    eca_b[:, 0:1], None, ALU.mult)
            nc.vector.scalar_tensor_tensor(band_sb[:], band3_sb[:, 64:128],
                                           eca_b[:, 1:2], band_sb[:],
                                           ALU.mult, ALU.add)
            nc.vector.scalar_tensor_tensor(band_sb[:], band3_sb[:, 128:192],
                                           eca_b[:, 2:3], band_sb[:],
                                           ALU.mult, ALU.add)

            def S1(b):
                """pass 1 streaming for block b: sum(x), sum(x^2)."""
                for gi in range(gpb):
                    g = b * gpb + gi
                    xt = xinp.tile([128, glen], F16, tag="x")
                    nc.sync.dma_start(xt[:],
                                      x_ext[:, g * glen:(g + 1) * glen])
                    sa = scrap.tile([128, glen], F16, tag="sa")
                    sd2 = scrdp.tile([128, glen], F16, tag="sd")
                    for k in range(GSEG):
                        s = g * GSEG + k
                        sk = s - b * BSEG      # index within block [0,16)
                        lo, hi = k * lp2, (k + 1) * lp2
                        if sk < P1_DVE:          # sum(x) on DVE
                            nc.vector.tensor_scalar(
                                sd2[:, lo:hi], xt[:, lo:hi], 0.0, None,
                                ALU.add, op1=ALU.add,
                                accum_out=s1_T[:, s:s + 1])
                        else:                    # sum(x) on Act (copy)
                            nc.scalar.activation(
                                sd2[:, lo:hi], xt[:, lo:hi], ACTF.Copy,
                                accum_out=s1_T[:, s:s + 1])
                        # sum(x^2) on Act
                        nc.scalar.activation(
                            sa[:, lo:hi], xt[:, lo:hi], ACTF.Square,
                            accum_out=s2_T[:, s:s + 1])

            def F1(b):
                """fold halves + AllReduce #1 for block b."""
                c0, c1 = b * BSEG, (b + 1) * BSEG
                fps = psp.tile([64, 2 * BSEG], F32, tag="fold1")
                nc.tensor.matmul(fps[:, 0:BSEG], ii_sb[:], s1_T[:, c0:c1],
                                 start=True, stop=True, skip_group_check=True)
                nc.tensor.matmul(fps[:, BSEG:2 * BSEG], ii_sb[:],
                                 s2_T[:, c0:c1],
                                 start=True, stop=True, skip_group_check=True)
                nc.scalar.copy(fold_sb[:, 2 * c0:2 * c1], fps[:])
                nc.sync.dma_start(ar1_in[b][:], fold_sb[:, 2 * c0:2 * c1])
                nc.gpsimd.collective_compute(
                    "AllReduce", ALU.add, replica_groups=rg,
                    ins=[ar1_in[b][:]], outs=[ar1_out[b][:]])
                nc.sync.dma_start(g1_sb[:, 2 * c0:2 * c1], ar1_out[b][:])

            def D1(b):
                """derive mu, rstd, packs for block b."""
                c0, c1 = b * BSEG, (b + 1) * BSEG
                ga = g1_sb[:, 2 * c0:2 * c0 + BSEG]
                gb = g1_sb[:, 2 * c0 + BSEG:2 * c1]
                nc.vector.tensor_scalar(mu_T[:, c0:c1], ga, rcnt, None,
                                        ALU.mult)
                nc.vector.tensor_scalar(es2[:, c0:c1], gb, rcnt, None,
                                        ALU.mult)
                nc.vector.tensor_tensor(var_T[:, c0:c1], mu_T[:, c0:c1],
                                        mu_T[:, c0:c1], ALU.mult)
                nc.vector.scalar_tensor_tensor(var_T[:, c0:c1],
                                               var_T[:, c0:c1], -1.0,
                                               es2[:, c0:c1],
                                               ALU.mult, ALU.add)
                nc.scalar.activation(sd_T[:, c0:c1], var_T[:, c0:c1],
                                     ACTF.Sqrt, bias=eps_col[:])
                nc.vector.reciprocal(rstd_T[:, c0:c1], sd_T[:, c0:c1])
                nc.vector.tensor_copy(mu_pack[0:64, c0:c1], mu_T[:, c0:c1])
                nc.sync.dma_start(mu_pack[64:128, c0:c1],
                                  mu_pack[0:64, c0:c1])
                nc.vector.tensor_scalar(nmu_pack[0:64, c0:c1],
                                        mu_T[:, c0:c1], -1.0, None, ALU.mult)
                nc.sync.dma_start(nmu_pack[64:128, c0:c1],
                                  nmu_pack[0:64, c0:c1])

            def S2(b):
                """pass 2 streaming for block b: sum(relu(x - mu))."""
                for gi in range(gpb):
                    g = b * gpb + gi
                    xt = xinp.tile([128, glen], F16, tag="x")
                    nc.sync.dma_start(xt[:],
                                      x_ext[:, g * glen:(g + 1) * glen])
                    sa = scrap.tile([128, glen], F16, tag="sa")
                    sd2 = scrdp.tile([128, glen], F16, tag="sd")
                    for k in range(GSEG):
                        s = g * GSEG + k
                        sk = s - b * BSEG
                        lo, hi = k * lp2, (k + 1) * lp2
                        if sk < P2_DVE:
                            # sum(max(x, mu)); corrected by -LP2*mu in C2F2
                            nc.vector.tensor_scalar(
                                sd2[:, lo:hi], xt[:, lo:hi],
                                mu_pack[:, s:s + 1], None,
                                ALU.max, op1=ALU.add,
                                accum_out=msC_T[:, s:s + 1])
                        else:
                            # relu(x - mu) summed directly on Act
                            nc.scalar.activation(
                                sa[:, lo:hi], xt[:, lo:hi], ACTF.Relu,
                                bias=nmu_pack[:, s:s + 1],
                                accum_out=msC_T[:, s:s + 1])

            def C2F2(b):
                """correct DVE relu-sums, fold, AllReduce #2 for block b."""
                c0, c1 = b * BSEG, (b + 1) * BSEG
                nc.vector.scalar_tensor_tensor(
                    msC_T[:, c0:c0 + P2_DVE], mu_pack[:, c0:c0 + P2_DVE],
                    float(-lp2), msC_T[:, c0:c0 + P2_DVE],
                    ALU.mult, ALU.add)
                fps = psp.tile([64, BSEG], F32, tag="fold2")
                nc.tensor.matmul(fps[:], ii_sb[:], msC_T[:, c0:c1],
                                 start=True, stop=True, skip_group_check=True)
                nc.scalar.copy(fold2_sb[:, c0:c1], fps[:])
                nc.sync.dma_start(ar2_in[b][:], fold2_sb[:, c0:c1])
                nc.gpsimd.collective_compute(
                    "AllReduce", ALU.add, replica_groups=rg,
                    ins=[ar2_in[b][:]], outs=[ar2_out[b][:]])
                nc.sync.dma_start(g2_sb[:, c0:c1], ar2_out[b][:])

            def D2(b):
                """m, ECA conv, sigmoid, S3/B3 packs for block b."""
                c0, c1 = b * BSEG, (b + 1) * BSEG
                nc.vector.tensor_scalar(m_T[:, c0:c1], g2_sb[:, c0:c1],
                                        rcnt, None, ALU.mult)
                nc.vector.tensor_tensor(m_T[:, c0:c1], m_T[:, c0:c1],
                                        rstd_T[:, c0:c1], ALU.mult)
                cps = psp.tile([64, BSEG], F32, tag="conv")
                nc.tensor.matmul(cps[:], band_sb[:], m_T[:, c0:c1],
                                 start=True, stop=True, skip_group_check=True)
                nc.scalar.activation(w_T[:, c0:c1], cps[:], ACTF.Sigmoid)
                nc.vector.tensor_tensor(s3_T[:, c0:c1], rstd_T[:, c0:c1],
                                        w_T[:, c0:c1], ALU.mult)
                nc.vector.scalar_tensor_tensor(b3_T[:, c0:c1], mu_T[:, c0:c1],
                                               -1.0, s3_T[:, c0:c1],
                                               ALU.mult, ALU.mult)
                nc.vector.tensor_copy(scale_pack[0:64, c0:c1], s3_T[:, c0:c1])
                nc.sync.dma_start(scale_pack[64:128, c0:c1],
                                  scale_pack[0:64, c0:c1])
                nc.vector.tensor_copy(bias_pack[0:64, c0:c1], b3_T[:, c0:c1])
                nc.sync.dma_start(bias_pack[64:128, c0:c1],
                                  bias_pack[0:64, c0:c1])

            def S3(b):
                """pass 3 streaming for block b: out = relu(x*S3 + B3)."""
                for gi in range(gpb):
                    g = b * gpb + gi
                    xt = xinp.tile([128, glen], F16, tag="x")
                    nc.sync.dma_start(xt[:],
                                      x_ext[:, g * glen:(g + 1) * glen])
                    ot = outp.tile([128, glen], F16, tag="o")
                    for k in range(GSEG):
                        s = g * GSEG + k
                        sk = s - b * BSEG
                        lo, hi = k * lp2, (k + 1) * lp2
                        if sk < P3_DVE:
                            nc.vector.tensor_scalar(
                                ot[:, lo:hi], xt[:, lo:hi],
                                scale_pack[:, s:s + 1],
                                bias_pack[:, s:s + 1], ALU.mult, op1=ALU.add)
                            nc.vector.tensor_scalar(
                                ot[:, lo:hi], ot[:, lo:hi], 0.0, None,
                                ALU.max)
                        else:
                            nc.scalar.activation(
                                ot[:, lo:hi], xt[:, lo:hi], ACTF.Relu,
                                bias=bias_pack[:, s:s + 1],
                                scale=scale_pack[:, s:s + 1])
                    nc.sync.dma_start(out_ext[:, g * glen:(g + 1) * glen],
                                      ot[:])

            # ---- pipelined schedule over blocks ----
            S1(0); F1(0)
            S1(1); F1(1)
            S1(2); F1(2)
            D1(0); S2(0); C2F2(0)
            S1(3); F1(3)
            D1(1); S2(1); C2F2(1)
            D2(0); S3(0)
            D1(2); S2(2); C2F2(2)
            D2(1); S3(1)
            D1(3); S2(3); C2F2(3)
            D2(2); S3(2)
            D2(3); S3(3)

    nc.compile()
    return nc


_cache = {}


def _get_nc(lp2):
    if lp2 not in _cache:
        _cache[lp2] = build_nc(lp2)
    return _cache[lp2]


last_result = None


def _install_ntff_hook():
    """Provide antenv.axon_hooks (missing in this image) so
    run_bass_kernel_spmd(trace=True) can reach the axon NTFF profiler."""
    import types

    try:
        from antenv.axon_hooks import get_axon_ntff_profile_hook  # noqa: F401
        return
    except ImportError:
        pass
    if "/root/.axon_site" not in sys.path:
        sys.path.insert(0, "/root/.axon_site")
    from trn_agent_boot.trn_boot import _ntff_profile_via_ctypes
    hook = _ntff_profile_via_ctypes("/opt/axon/libaxon_pjrt.so")
    try:
        import antenv
    except ImportError:
        antenv = types.ModuleType("antenv")
        sys.modules["antenv"] = antenv
    mod = types.ModuleType("antenv.axon_hooks")
    mod.get_axon_ntff_profile_hook = lambda: hook
    mod.set_axon_ntff_profile_hook = lambda h: None
    sys.modules["antenv.axon_hooks"] = mod
    antenv.axon_hooks = mod
    import concourse.bass_utils as _bu
    _bu.upload_artifacts = lambda d: "local://" + str(d)


def _prep_inputs(x, idx, eca):
    """Sort by segment, deal evenly over cores, pad each (core, seg) run
    to the common even length LP with duplicated points."""
    order = np.argsort(idx, kind="stable")
    counts = np.bincount(idx, minlength=NSEG).astype(np.int64)
    starts = np.zeros(NSEG + 1, np.int64)
    starts[1:] = np.cumsum(counts)

    q, r = np.divmod(counts, NCORES)
    maxchunk = int((q + (r > 0).astype(np.int64)).max())
    lp = max(2, ((maxchunk + 1) // 2) * 2)       # even
    lp2 = lp // 2

    grids = []
    in_maps = []
    for kcore in range(NCORES):
        grid = np.empty((NSEG, lp), np.int64)
        for s in range(NSEG):
            n_s = counts[s]
            run = order[starts[s]:starts[s] + n_s]
            qq, rr = divmod(int(n_s), NCORES)
            a = kcore * qq + min(kcore, rr)
            b = a + qq + (1 if kcore < rr else 0)
            chunk = run[a:b]
            assert chunk.size > 0, f"empty (core,seg)=({kcore},{s})"
            grid[s] = np.resize(chunk, lp)
        grids.append(grid)
        xg = x[grid.reshape(-1)].reshape(NSEG, lp2, 2, C)
        dev = np.ascontiguousarray(
            xg.transpose(2, 3, 0, 1).reshape(128, NSEG * lp2),
            dtype=np.float16)
        in_maps.append({"xT": dev, "eca_weight": eca})
    return in_maps, grids, lp2


def kernel(features, ins_indices_batch, eca_weight, _trace=False):
    global last_result
    x = np.asarray(features, np.float32)
    idx = np.asarray(ins_indices_batch, np.int32)
    eca = np.asarray(eca_weight, np.float32).reshape(1, 3)
    n = x.shape[0]

    in_maps, grids, lp2 = _prep_inputs(x, idx, eca)
    nc = _get_nc(lp2)

    if _trace:
        _install_ntff_hook()
    try:
        res = run_bass_kernel_spmd(nc, in_maps, core_ids=list(range(NCORES)),
                                   trace=_trace)
    except Exception:
        if not _trace:
            raise
        import traceback
        traceback.print_exc()
        print("traced run failed; falling back to untraced", flush=True)
        res = run_bass_kernel_spmd(nc, in_maps, core_ids=list(range(NCORES)))
    last_result = res

    out = np.empty((n, C), np.float32)
    lp = 2 * lp2
    for kcore in range(NCORES):
        od = res.results[kcore]["out"]            # [128, NSEG*lp2] f16
        vals = od.reshape(2, C, NSEG, lp2).transpose(2, 3, 0, 1)
        out[grids[kcore].reshape(-1)] = vals.reshape(NSEG * lp, C)
    return out


if __name__ == "__main__":
    rng = np.random.default_rng(0)
    n_test = 200_000
    x = rng.standard_normal((n_test, C), dtype=np.float32)
    ii = rng.integers(0, NSEG, n_test).astype(np.int32)
    k = (rng.standard_normal((1, 1, 3)) * 0.1).astype(np.float32)
    out = kernel(x, ii, k)

    seg = ii
    cnt = np.maximum(np.bincount(seg, minlength=NSEG), 1).astype(np.float64)
    s = np.zeros((NSEG, C)); np.add.at(s, seg, x.astype(np.float64))
    s2 = np.zeros((NSEG, C)); np.add.at(s2, seg, x.astype(np.float64) ** 2)
    mu = s / cnt[:, None]
    var = s2 / cnt[:, None] - mu ** 2
    xn = (x - mu[seg]) / np.sqrt(var[seg] + EPS)
    xr = np.maximum(xn, 0)
    m = np.zeros((NSEG, C)); np.add.at(m, seg, xr)
    m = m / cnt[:, None]
    kf = k.reshape(3)
    mp = np.pad(m, ((0, 0), (1, 1)))
    conv = kf[0] * mp[:, 0:64] + kf[1] * mp[:, 1:65] + kf[2] * mp[:, 2:66]
    w = 1.0 / (1.0 + np.exp(-conv))
    exp = xr * w[seg]
    err = np.linalg.norm(out - exp) / np.linalg.norm(exp)
    print("out", out.shape, out.dtype, "rel_err", err)


# revision 18
# speedup vs baseline: 3.0075x; 1.1254x over previous
"""Trainium2 Bass kernel for DensePose sparse GN head (segment_reduce).

out = relu((x - mu[seg]) * rstd[seg]) * sigmoid(conv1d(segmean(relu(xn))))[seg]

Host pre-sorts points by segment id and deals them evenly across the 8
cores so every (core, segment) run has identical length LP (padded with
duplicated points; counts become the compile-time constant 8*LP).  Data
is f16, channel-major, two point-halves packed on 128 partitions:

    x_dev[half*64 + ch, s*LP2 + t] = x[pt(core, s, 2*t + half), ch]

Every segment is a contiguous column range: segment sums are free-dim
reductions (DVE tensor_tensor_reduce / Act accumulate), normalize+ECA
scaling is per-partition scale/bias.  The ECA conv over channels
(= partitions) is a tiny tridiagonal-band 64x64 PE matmul.

The computation is separable per segment, so segments are processed in
NBLK pipelined blocks: pass1(b) -> AllReduce1(b) -> pass2(b) ->
AllReduce2(b) -> pass3(b), with blocks interleaved so the collectives
hide behind other blocks' streaming.

Identities (w > 0, rstd > 0):
  out = relu(x*S3 + B3),  S3 = rstd*w,  B3 = -mu*rstd*w
  segsum(relu(x - mu)) = segsum(max(x, mu)) - cnt*mu
"""

import sys

for _p in ("/opt/trn_rl_repo",):
    if _p not in sys.path:
        sys.path.append(_p)

import numpy as np

import concourse.bass as bass
import concourse.bacc as bacc
import concourse.mybir as mybir
import concourse.tile as tile
from concourse.bass_utils import run_bass_kernel_spmd

F32 = mybir.dt.float32
F16 = mybir.dt.float16
ALU = mybir.AluOpType
ACTF = mybir.ActivationFunctionType

C = 64
NSEG = 64
EPS = 1e-5
NCORES = 8
GSEG = 4           # segments per DMA group
NBLK = 4           # pipeline blocks
BSEG = NSEG // NBLK   # segments per block (16)

# engine split knobs (per 16-seg block); sum(x^2) always runs on Act
P1_DVE = 16        # segs whose sum(x) runs on DVE (rest on Act copy+accum)
P2_DVE = 5         # segs whose relu-sum runs on DVE (rest on Act relu+accum)
P3_DVE = 16        # segs whose output op runs on DVE (rest on Act relu)
NRES = 6           # trailing DMA groups kept resident in SBUF (of 16)


def _consts(nc):
    ii = np.vstack([np.eye(64, dtype=np.float32)] * 2)          # [128, 64]
    band3 = np.zeros((64, 192), np.float32)
    for j, off in enumerate((-1, 0, 1)):
        for c in range(64):
            cp = c + off
            if 0 <= cp < 64:
                band3[cp, 64 * j + c] = 1.0
    ones_row = np.ones((1, 64), np.float32)
    c = {}
    c["ii"] = nc.inline_tensor(np.ascontiguousarray(ii), name="ii_f")
    c["band3"] = nc.inline_tensor(np.ascontiguousarray(band3), name="band3_f")
    c["ones_row"] = nc.inline_tensor(ones_row, name="ones_row_f")
    return c


def build_nc(lp2):
    ncols = NSEG * lp2
    glen = GSEG * lp2
    blen = BSEG * lp2
    gpb = BSEG // GSEG           # DMA groups per block
    rcnt = 1.0 / float(NCORES * 2 * lp2)     # 1 / (8 * LP)

    nc = bacc.Bacc("TRN2", target_bir_lowering=False, debug=False,
                   num_devices=NCORES)
    x_ext = nc.declare_dram_parameter("xT", [128, ncols], F16, isOutput=False)
    eca_ext = nc.declare_dram_parameter("eca_weight", [1, 3], F32,
                                        isOutput=False)
    out_ext = nc.declare_dram_parameter("out", [128, ncols], F16,
                                        isOutput=True)
    cst = _consts(nc)
    rg = [list(range(NCORES))]

    with tile.TileContext(nc) as tc:
        with (
            tc.tile_pool(name="const", bufs=1) as constp,
            tc.tile_pool(name="dram", bufs=1, space="DRAM") as dramp,
            tc.tile_pool(name="small", bufs=1) as smallp,
            tc.tile_pool(name="xin", bufs=3) as xinp,
            tc.tile_pool(name="res", bufs=1) as resp,
            tc.tile_pool(name="outp", bufs=2) as outp,
            tc.tile_pool(name="ps", bufs=2, space="PSUM") as psp,
        ):
            # ---- constants ----
            ii_sb = constp.tile([128, 64], F32)
            band3_sb = constp.tile([64, 192], F32)
            ones_sb = constp.tile([1, 64], F32)
            eca_sb = constp.tile([1, 3], F32)
            nc.sync.dma_start(ii_sb[:], cst["ii"][:, :])
            nc.sync.dma_start(band3_sb[:], cst["band3"][:, :])
            nc.sync.dma_start(ones_sb[:], cst["ones_row"][:, :])
            nc.sync.dma_start(eca_sb[:], eca_ext[:, :])

            # ---- collective bounce buffers (per block) ----
            ar1_in = [dramp.tile([64, 2 * BSEG], F32, name=f"ar1_in{b}")
                      for b in range(NBLK)]
            ar1_out = [dramp.tile([64, 2 * BSEG], F32, addr_space="Shared",
                                  name=f"ar1_out{b}") for b in range(NBLK)]
            ar2_in = [dramp.tile([64, BSEG], F32, name=f"ar2_in{b}")
                      for b in range(NBLK)]
            ar2_out = [dramp.tile([64, BSEG], F32, addr_space="Shared",
                                  name=f"ar2_out{b}") for b in range(NBLK)]

            # ---- persistent small tensors ----
            s1_T = smallp.tile([128, 64], F32)    # sum(x) per (half*ch, seg)
            s2_T = smallp.tile([128, 64], F32)    # sum(x^2)
            msC_T = smallp.tile([128, 64], F32)   # relu-sum (corrected)
            g1_sb = smallp.tile([64, 2 * 64], F32)
            g2_sb = smallp.tile([64, 64], F32)
            fold_sb = smallp.tile([64, 2 * 64], F32)
            fold2_sb = smallp.tile([64, 64], F32)
            mu_T = smallp.tile([64, 64], F32)     # [ch, seg]
            es2 = smallp.tile([64, 64], F32)
            var_T = smallp.tile([64, 64], F32)
            sd_T = smallp.tile([64, 64], F32)
            rstd_T = smallp.tile([64, 64], F32)
            m_T = smallp.tile([64, 64], F32)
            w_T = smallp.tile([64, 64], F32)
            s3_T = smallp.tile([64, 64], F32)
            b3_T = smallp.tile([64, 64], F32)
            band_sb = smallp.tile([64, 64], F32)
            eca_b = smallp.tile([64, 3], F32)
            mu_pack = smallp.tile([128, 64], F32)
            nmu_pack = smallp.tile([128, 64], F32)
            scale_pack = smallp.tile([128, 64], F32)
            bias_pack = smallp.tile([128, 64], F32)
            eps_col = smallp.tile([64, 1], F32)
            nc.vector.memset(eps_col[:], float(EPS))
            # write-only scratch (one per engine; WAW on same queue is free)
            sa_scr = smallp.tile([128, lp2], F16)
            sd_scr = smallp.tile([128, lp2], F16)

            G_RES = NSEG // GSEG - NRES     # groups >= G_RES stay resident
            res_tiles = {}

            def load_group(g):
                """DMA group g into a tile; resident groups load once."""
                if g >= G_RES:
                    if g in res_tiles:
                        return res_tiles[g]
                    xt = resp.tile([128, glen], F16, tag=f"res{g}",
                                   name=f"res{g}")
                    res_tiles[g] = xt
                else:
                    xt = xinp.tile([128, glen], F16, tag="x", name="xt")
                nc.sync.dma_start(xt[:], x_ext[:, g * glen:(g + 1) * glen])
                return xt

            def get_group(g):
                if g >= G_RES:
                    return res_tiles[g]
                return load_group(g)

            # ---- ECA weight broadcast + band matrix (input-only deps) ----
            eca_ps = psp.tile([64, 3], F32, tag="eca")
            nc.tensor.matmul(eca_ps[:], ones_sb[:], eca_sb[:],
                             start=True, stop=True, skip_group_check=True)
            nc.scalar.copy(eca_b[:], eca_ps[:])
            nc.vector.tensor_scalar(band_sb[:], band3_sb[:, 0:64],
                                    eca_b[:, 0:1], None, ALU.mult)
            nc.vector.scalar_tensor_tensor(band_sb[:], band3_sb[:, 64:128],
                                           eca_b[:, 1:2], band_sb[:],
                                           ALU.mult, ALU.add)
            nc.vector.scalar_tensor_tensor(band_sb[:], band3_sb[:, 128:192],
                                           eca_b[:, 2:3], band_sb[:],
                                           ALU.mult, ALU.add)

            def S1(b):
                """pass 1 streaming for block b: sum(x), sum(x^2)."""
                for gi in range(gpb):
                    g = b * gpb + gi
                    xt = load_group(g)
                    for k in range(GSEG):
                        s = g * GSEG + k
                        sk = s - b * BSEG      # index within block [0,16)
                        lo, hi = k * lp2, (k + 1) * lp2
                        if sk < P1_DVE:          # sum(x) on DVE
                            nc.vector.tensor_scalar(
                                sd_scr[:], xt[:, lo:hi], 0.0, None,
                                ALU.add, op1=ALU.add,
                                accum_out=s1_T[:, s:s + 1])
                        else:                    # sum(x) on Act (copy)
                            nc.scalar.activation(
                                sa_scr[:], xt[:, lo:hi], ACTF.Copy,
                                accum_out=s1_T[:, s:s + 1])
                        # sum(x^2) on Act
                        nc.scalar.activation(
                            sa_scr[:], xt[:, lo:hi], ACTF.Square,
                            accum_out=s2_T[:, s:s + 1])

            def F1(b):
                """fold halves + AllReduce #1 for block b."""
                c0, c1 = b * BSEG, (b + 1) * BSEG
                fps = psp.tile([64, 2 * BSEG], F32, tag="fold1")
                nc.tensor.matmul(fps[:, 0:BSEG], ii_sb[:], s1_T[:, c0:c1],
                                 start=True, stop=True, skip_group_check=True)
                nc.tensor.matmul(fps[:, BSEG:2 * BSEG], ii_sb[:],
                                 s2_T[:, c0:c1],
                                 start=True, stop=True, skip_group_check=True)
                nc.scalar.copy(fold_sb[:, 2 * c0:2 * c1], fps[:])
                nc.sync.dma_start(ar1_in[b][:], fold_sb[:, 2 * c0:2 * c1])
                nc.gpsimd.collective_compute(
                    "AllReduce", ALU.add, replica_groups=rg,
                    ins=[ar1_in[b][:]], outs=[ar1_out[b][:]])
                nc.sync.dma_start(g1_sb[:, 2 * c0:2 * c1], ar1_out[b][:])

            def D1(b):
                """derive mu, rstd, packs for block b."""
                c0, c1 = b * BSEG, (b + 1) * BSEG
                ga = g1_sb[:, 2 * c0:2 * c0 + BSEG]
                gb = g1_sb[:, 2 * c0 + BSEG:2 * c1]
                nc.vector.tensor_scalar(mu_T[:, c0:c1], ga, rcnt, None,
                                        ALU.mult)
                nc.vector.tensor_scalar(es2[:, c0:c1], gb, rcnt, None,
                                        ALU.mult)
                nc.vector.tensor_tensor(var_T[:, c0:c1], mu_T[:, c0:c1],
                                        mu_T[:, c0:c1], ALU.mult)
                nc.vector.scalar_tensor_tensor(var_T[:, c0:c1],
                                               var_T[:, c0:c1], -1.0,
                                               es2[:, c0:c1],
                                               ALU.mult, ALU.add)
                nc.scalar.activation(sd_T[:, c0:c1], var_T[:, c0:c1],
                                     ACTF.Sqrt, bias=eps_col[:])
                nc.vector.reciprocal(rstd_T[:, c0:c1], sd_T[:, c0:c1])
                nc.vector.tensor_copy(mu_pack[0:64, c0:c1], mu_T[:, c0:c1])
                nc.sync.dma_start(mu_pack[64:128, c0:c1],
                                  mu_pack[0:64, c0:c1])
                nc.vector.tensor_scalar(nmu_pack[0:64, c0:c1],
                                        mu_T[:, c0:c1], -1.0, None, ALU.mult)
                nc.sync.dma_start(nmu_pack[64:128, c0:c1],
                                  nmu_pack[0:64, c0:c1])

            def S2(b):
                """pass 2 streaming for block b: sum(relu(x - mu))."""
                for gi in range(gpb):
                    g = b * gpb + gi
                    xt = get_group(g)
                    for k in range(GSEG):
                        s = g * GSEG + k
                        sk = s - b * BSEG
                        lo, hi = k * lp2, (k + 1) * lp2
                        if sk < P2_DVE:
                            # sum(max(x, mu)); corrected by -LP2*mu in C2F2
                            nc.vector.tensor_scalar(
                                sd_scr[:], xt[:, lo:hi],
                                mu_pack[:, s:s + 1], None,
                                ALU.max, op1=ALU.add,
                                accum_out=msC_T[:, s:s + 1])
                        else:
                            # relu(x - mu) summed directly on Act
                            nc.scalar.activation(
                                sa_scr[:], xt[:, lo:hi], ACTF.Relu,
                                bias=nmu_pack[:, s:s + 1],
                                accum_out=msC_T[:, s:s + 1])

            def C2F2(b):
                """correct DVE relu-sums, fold, AllReduce #2 for block b."""
                c0, c1 = b * BSEG, (b + 1) * BSEG
                nc.vector.scalar_tensor_tensor(
                    msC_T[:, c0:c0 + P2_DVE], mu_pack[:, c0:c0 + P2_DVE],
                    float(-lp2), msC_T[:, c0:c0 + P2_DVE],
                    ALU.mult, ALU.add)
                fps = psp.tile([64, BSEG], F32, tag="fold2")
                nc.tensor.matmul(fps[:], ii_sb[:], msC_T[:, c0:c1],
                                 start=True, stop=True, skip_group_check=True)
                nc.scalar.copy(fold2_sb[:, c0:c1], fps[:])
                nc.sync.dma_start(ar2_in[b][:], fold2_sb[:, c0:c1])
                nc.gpsimd.collective_compute(
                    "AllReduce", ALU.add, replica_groups=rg,
                    ins=[ar2_in[b][:]], outs=[ar2_out[b][:]])
                nc.sync.dma_start(g2_sb[:, c0:c1], ar2_out[b][:])

            def D2(b):
                """m, ECA conv, sigmoid, S3/B3 packs for block b."""
                c0, c1 = b * BSEG, (b + 1) * BSEG
                nc.vector.tensor_scalar(m_T[:, c0:c1], g2_sb[:, c0:c1],
                                        rcnt, None, ALU.mult)
                nc.vector.tensor_tensor(m_T[:, c0:c1], m_T[:, c0:c1],
                                        rstd_T[:, c0:c1], ALU.mult)
                cps = psp.tile([64, BSEG], F32, tag="conv")
                nc.tensor.matmul(cps[:], band_sb[:], m_T[:, c0:c1],
                                 start=True, stop=True, skip_group_check=True)
                nc.scalar.activation(w_T[:, c0:c1], cps[:], ACTF.Sigmoid)
                nc.vector.tensor_tensor(s3_T[:, c0:c1], rstd_T[:, c0:c1],
                                        w_T[:, c0:c1], ALU.mult)
                nc.vector.scalar_tensor_tensor(b3_T[:, c0:c1], mu_T[:, c0:c1],
                                               -1.0, s3_T[:, c0:c1],
                                               ALU.mult, ALU.mult)
                nc.vector.tensor_copy(scale_pack[0:64, c0:c1], s3_T[:, c0:c1])
                nc.sync.dma_start(scale_pack[64:128, c0:c1],
                                  scale_pack[0:64, c0:c1])
                nc.vector.tensor_copy(bias_pack[0:64, c0:c1], b3_T[:, c0:c1])
                nc.sync.dma_start(bias_pack[64:128, c0:c1],
                                  bias_pack[0:64, c0:c1])

            def S3(b):
                """pass 3 streaming for block b: out = relu(x*S3 + B3)."""
                for gi in range(gpb):
                    g = b * gpb + gi
                    xt = get_group(g)
                    ot = outp.tile([128, glen], F16, tag="o")
                    for k in range(GSEG):
                        s = g * GSEG + k
                        sk = s - b * BSEG
                        lo, hi = k * lp2, (k + 1) * lp2
                        if sk < P3_DVE:
                            nc.vector.tensor_scalar(
                                ot[:, lo:hi], xt[:, lo:hi],
                                scale_pack[:, s:s + 1],
                                bias_pack[:, s:s + 1], ALU.mult, op1=ALU.add)
                            nc.vector.tensor_scalar(
                                ot[:, lo:hi], ot[:, lo:hi], 0.0, None,
                                ALU.max)
                        else:
                            nc.scalar.activation(
                                ot[:, lo:hi], xt[:, lo:hi], ACTF.Relu,
                                bias=bias_pack[:, s:s + 1],
                                scale=scale_pack[:, s:s + 1])
                    nc.sync.dma_start(out_ext[:, g * glen:(g + 1) * glen],
                                      ot[:])

            # ---- pipelined schedule over blocks ----
            S1(0); F1(0)
            S1(1); F1(1)
            S1(2); F1(2)
            D1(0); S2(0); C2F2(0)
            S1(3); F1(3)
            D1(1); S2(1); C2F2(1)
            D2(0); S3(0)
            D1(2); S2(2); C2F2(2)
            D2(1); S3(1)
            D1(3); S2(3); C2F2(3)
            D2(2); S3(2)
            D2(3); S3(3)

    nc.compile()
    return nc


_cache = {}


def _get_nc(lp2):
    if lp2 not in _cache:
        _cache[lp2] = build_nc(lp2)
    return _cache[lp2]


last_result = None


def _install_ntff_hook():
    """Provide antenv.axon_hooks (missing in this image) so
    run_bass_kernel_spmd(trace=True) can reach the axon NTFF profiler."""
    import types

    try:
        from antenv.axon_hooks import get_axon_ntff_profile_hook  # noqa: F401
        return
    except ImportError:
        pass
    if "/root/.axon_site" not in sys.path:
        sys.path.insert(0, "/root/.axon_site")
    from trn_agent_boot.trn_boot import _ntff_profile_via_ctypes
    hook = _ntff_profile_via_ctypes("/opt/axon/libaxon_pjrt.so")
    try:
        import antenv
    except ImportError:
        antenv = types.ModuleType("antenv")
        sys.modules["antenv"] = antenv
    mod = types.ModuleType("antenv.axon_hooks")
    mod.get_axon_ntff_profile_hook = lambda: hook
    mod.set_axon_ntff_profile_hook = lambda h: None
    sys.modules["antenv.axon_hooks"] = mod
    antenv.axon_hooks = mod
    import concourse.bass_utils as _bu
    _bu.upload_artifacts = lambda d: "local://" + str(d)


def _prep_inputs(x, idx, eca):
    """Sort by segment, deal evenly over cores, pad each (core, seg) run
    to the common even length LP with duplicated points."""
    order = np.argsort(idx, kind="stable")
    counts = np.bincount(idx, minlength=NSEG).astype(np.int64)
    starts = np.zeros(NSEG + 1, np.int64)
    starts[1:] = np.cumsum(counts)

    q, r = np.divmod(counts, NCORES)
    maxchunk = int((q + (r > 0).astype(np.int64)).max())
    lp = max(2, ((maxchunk + 1) // 2) * 2)       # even
    lp2 = lp // 2

    grids = []
    in_maps = []
    for kcore in range(NCORES):
        grid = np.empty((NSEG, lp), np.int64)
        for s in range(NSEG):
            n_s = counts[s]
            run = order[starts[s]:starts[s] + n_s]
            qq, rr = divmod(int(n_s), NCORES)
            a = kcore * qq + min(kcore, rr)
            b = a + qq + (1 if kcore < rr else 0)
            chunk = run[a:b]
            assert chunk.size > 0, f"empty (core,seg)=({kcore},{s})"
            grid[s] = np.resize(chunk, lp)
        grids.append(grid)
        xg = x[grid.reshape(-1)].reshape(NSEG, lp2, 2, C)
        dev = np.ascontiguousarray(
            xg.transpose(2, 3, 0, 1).reshape(128, NSEG * lp2),
            dtype=np.float16)
        in_maps.append({"xT": dev, "eca_weight": eca})
    return in_maps, grids, lp2


def kernel(features, ins_indices_batch, eca_weight, _trace=False):
    global last_result
    x = np.asarray(features, np.float32)
    idx = np.asarray(ins_indices_batch, np.int32)
    eca = np.asarray(eca_weight, np.float32).reshape(1, 3)
    n = x.shape[0]

    in_maps, grids, lp2 = _prep_inputs(x, idx, eca)
    nc = _get_nc(lp2)

    if _trace:
        _install_ntff_hook()
    try:
        res = run_bass_kernel_spmd(nc, in_maps, core_ids=list(range(NCORES)),
                                   trace=_trace)
    except Exception:
        if not _trace:
            raise
        import traceback
        traceback.print_exc()
        print("traced run failed; falling back to untraced", flush=True)
        res = run_bass_kernel_spmd(nc, in_maps, core_ids=list(range(NCORES)))
    last_result = res

    out = np.empty((n, C), np.float32)
    lp = 2 * lp2
    for kcore in range(NCORES):
        od = res.results[kcore]["out"]            # [128, NSEG*lp2] f16
        vals = od.reshape(2, C, NSEG, lp2).transpose(2, 3, 0, 1)
        out[grids[kcore].reshape(-1)] = vals.reshape(NSEG * lp, C)
    return out


if __name__ == "__main__":
    rng = np.random.default_rng(0)
    n_test = 200_000
    x = rng.standard_normal((n_test, C), dtype=np.float32)
    ii = rng.integers(0, NSEG, n_test).astype(np.int32)
    k = (rng.standard_normal((1, 1, 3)) * 0.1).astype(np.float32)
    out = kernel(x, ii, k)

    seg = ii
    cnt = np.maximum(np.bincount(seg, minlength=NSEG), 1).astype(np.float64)
    s = np.zeros((NSEG, C)); np.add.at(s, seg, x.astype(np.float64))
    s2 = np.zeros((NSEG, C)); np.add.at(s2, seg, x.astype(np.float64) ** 2)
    mu = s / cnt[:, None]
    var = s2 / cnt[:, None] - mu ** 2
    xn = (x - mu[seg]) / np.sqrt(var[seg] + EPS)
    xr = np.maximum(xn, 0)
    m = np.zeros((NSEG, C)); np.add.at(m, seg, xr)
    m = m / cnt[:, None]
    kf = k.reshape(3)
    mp = np.pad(m, ((0, 0), (1, 1)))
    conv = kf[0] * mp[:, 0:64] + kf[1] * mp[:, 1:65] + kf[2] * mp[:, 2:66]
    w = 1.0 / (1.0 + np.exp(-conv))
    exp = xr * w[seg]
    err = np.linalg.norm(out - exp) / np.linalg.norm(exp)
    print("out", out.shape, out.dtype, "rel_err", err)
